# revision 1
# baseline (speedup 1.0000x reference)
"""Chamfer-distance criterion kernel for Trainium2 (8 NeuronCores, data-parallel over batch).

Math: the reference's two [B,T,T] pairwise cross-entropy GEMMs collapse exactly
because one side of each GEMM is a (masked) one-hot:

  probs = softmax(logits); p0 = probs[:,:,0]; valid = (t!=0)&(t!=PAD)
  knn_ce(one_hot, xs) = sum_{valid n} -log(clip(max_m probs[b,m,t_n]*valid_m))
  knn_ce(xs, one_hot) = sum_n valid_n*(C0*(1-p0) - (C0-C1)*max_{valid j} probs[b,n,t_j])
     with C0 = -log(eps), C1 = -log(1-(D-1)*eps)

So each core only needs, per row: Z = sum(exp(l)) and the logits gathered at
its batch's (<=64) target columns + column 0.  That is one streaming pass over
the logits (memory-bound) + tiny reductions, done fully on device; the host
just sums the 8 per-core partial scalars.
"""

import math
import numpy as np
from contextlib import ExitStack

import concourse.bass as bass
import concourse.tile as tile
from concourse import bacc, mybir
from concourse import library_config

# ---- problem constants (hardcoded per contract) ----
B, T, V = 64, 64, 8192
PAD = 8192
EPS = 1e-8
D = V - 1
C0 = float(-math.log(EPS))
C1 = float(-math.log1p(-(D - 1) * EPS))
HI = float(1.0 - (D - 1) * EPS)

N_CORES = 8
BPC = B // N_CORES          # batches per core = 8
ROWS = BPC * T              # rows per core = 512
P = 128                     # partitions per tile
NT = ROWS // P              # tiles per core = 4
NJ = 64                     # gather slots: one per target position
F32 = mybir.dt.float32
I16 = mybir.dt.int16


def _build_program(reps=1):
    nc = bacc.Bacc("TRN2", target_bir_lowering=False, debug=False)
    x_d = nc.dram_tensor("x", [ROWS, V], F32, kind="ExternalInput").ap()
    idx_d = nc.dram_tensor("idx", [P, 4 * NT], I16, kind="ExternalInput").ap()
    mj_d = nc.dram_tensor("mj", [P, NJ * NT], F32, kind="ExternalInput").ap()
    mval_d = nc.dram_tensor("mval", [P, NT], F32, kind="ExternalInput").ap()
    epw_d = nc.dram_tensor("epw", [P, NT], F32, kind="ExternalInput").ap()
    ehw_d = nc.dram_tensor("ehw", [P, NT], F32, kind="ExternalInput").ap()
    mvt_d = nc.dram_tensor("mvt", [64, BPC], F32, kind="ExternalInput").ap()
    ident_d = nc.dram_tensor("ident", [P, 64], F32, kind="ExternalInput").ap()
    ones_d = nc.dram_tensor("ones", [P, 1], F32, kind="ExternalInput").ap()
    out_d = nc.dram_tensor("out", [1, 2], F32, kind="ExternalOutput").ap()

    AF = mybir.ActivationFunctionType
    OP = mybir.AluOpType

    with tile.TileContext(nc) as tc:
        with ExitStack() as ctx:
            const = ctx.enter_context(tc.tile_pool(name="const", bufs=1))
            xp = ctx.enter_context(tc.tile_pool(name="xp", bufs=3))
            epool = ctx.enter_context(tc.tile_pool(name="ep", bufs=2))
            gp = ctx.enter_context(tc.tile_pool(name="gp", bufs=NT))
            pgp = ctx.enter_context(tc.tile_pool(name="pgp", bufs=2))
            rowp = ctx.enter_context(tc.tile_pool(name="rowp", bufs=2))
            small = ctx.enter_context(tc.tile_pool(name="small", bufs=1))
            tpp = ctx.enter_context(tc.tile_pool(name="tpp", bufs=4, space="PSUM"))
            finp = ctx.enter_context(tc.tile_pool(name="finp", bufs=1, space="PSUM"))

            # gpsimd ext-isa library for ap_gather; emit before any gather
            nc.gpsimd.load_library(library_config.ap_gather)

            # constants / marshaled inputs
            ident = const.tile([P, 64], F32)
            nc.sync.dma_start(ident[:], ident_d[:])
            ones = const.tile([P, 1], F32)
            nc.sync.dma_start(ones[:], ones_d[:])
            mval = const.tile([P, NT], F32)
            nc.sync.dma_start(mval[:], mval_d[:])
            epw = const.tile([P, NT], F32)
            nc.sync.dma_start(epw[:], epw_d[:])
            ehw = const.tile([P, NT], F32)
            nc.sync.dma_start(ehw[:], ehw_d[:])
            mvt = const.tile([64, BPC], F32)
            nc.sync.dma_start(mvt[:], mvt_d[:])
            mj = const.tile([P, NJ * NT], F32)
            nc.sync.dma_start(mj[:], mj_d[:])

            for rep in range(reps):
                zcol = small.tile([P, NT], F32, tag="zcol")
                lzneg = small.tile([P, NT], F32, tag="lzneg")
                p0col = small.tile([P, NT], F32, tag="p0col")
                l0col = small.tile([P, NT], F32, tag="l0col")
                m2col = small.tile([P, NT], F32, tag="m2col")
                m1all = small.tile([64, BPC], F32, tag="m1all")
                rcat = small.tile([P, 3 * NT], F32, tag="rcat")
                t1col = small.tile([64, BPC], F32, tag="t1col")

                gts = []
                # ---- streaming pass: exp+rowsum and gather per [128, V] tile ----
                for i in range(NT):
                    xt = xp.tile([P, V], F32, tag="xt")
                    nc.sync.dma_start(xt[:], x_d[i * P:(i + 1) * P, :])
                    et = epool.tile([P, V], F32, tag="et")
                    nc.scalar.activation(et[:], xt[:], AF.Exp,
                                         accum_out=zcol[:, i:i + 1])
                    # ap_gather needs its idx operand as a whole tile (the Q7
                    # ucode mishandles free-dim AP offsets on the idx input)
                    it = const.tile([P, 4], I16, tag=f"it{i}")
                    nc.sync.dma_start(it[:], idx_d[:, 4 * i:4 * (i + 1)])
                    nc.vector.tensor_copy(l0col[:, i:i + 1], xt[:, 0:1])
                    gt = gp.tile([P, NJ], F32, tag="gt")
                    nc.gpsimd.ap_gather(gt[:], xt[:], it[:],
                                        channels=P, num_elems=V, d=1, num_idxs=NJ)
                    gts.append(gt)

                # ---- per-tile epilogue ----
                nc.scalar.activation(lzneg[:], zcol[:], AF.Ln)
                nc.scalar.mul(lzneg[:], lzneg[:], -1.0)
                for i in range(NT):
                    gt = gts[i]
                    pg = pgp.tile([P, NJ], F32, tag="pg")
                    nc.scalar.activation(pg[:], gt[:], AF.Exp,
                                         bias=lzneg[:, i:i + 1], scale=1.0)
                    pgm = pgp.tile([P, NJ], F32, tag="pgm")
                    nc.vector.tensor_mul(pgm[:], pg[:], mj[:, NJ * i:NJ * (i + 1)])
                    nc.vector.tensor_reduce(m2col[:, i:i + 1], pgm[:],
                                            axis=mybir.AxisListType.X, op=OP.max)
                    pgrow = rowp.tile([P, 64], F32, tag="pgrow")
                    nc.vector.tensor_scalar_mul(pgrow[:], pg[:, 0:64],
                                                scalar1=mval[:, i:i + 1])
                    nc.scalar.activation(p0col[:, i:i + 1], l0col[:, i:i + 1],
                                         AF.Exp, bias=lzneg[:, i:i + 1], scale=1.0)
                    for h in range(2):
                        b = 2 * i + h
                        tp = tpp.tile([64, 64], F32, tag="tp")
                        nc.tensor.transpose(tp[:], pgrow[64 * h:64 * h + 64, :],
                                            ident[64 * h:64 * h + 64, :])
                        nc.vector.tensor_reduce(m1all[:, b:b + 1], tp[:],
                                                axis=mybir.AxisListType.X, op=OP.max)

                # ---- wide epilogue ([P, NT] / [64, BPC] shapes) ----
                logp0 = small.tile([P, NT], F32, tag="logp0")
                nc.vector.tensor_add(logp0[:], l0col[:], lzneg[:])
                onem = small.tile([P, NT], F32, tag="onem")
                nc.scalar.activation(onem[:], p0col[:], AF.Copy, scale=-1.0, bias=1.0)
                log1m = small.tile([P, NT], F32, tag="log1m")
                nc.scalar.activation(log1m[:], onem[:], AF.Ln)
                c0t = small.tile([P, NT], F32, tag="c0t")
                nc.scalar.activation(c0t[:], p0col[:], AF.Copy, scale=-C0, bias=C0)
                tmp2 = small.tile([P, NT], F32, tag="tmp2")
                # tmp2 = c0t - (C0-C1)*m2col
                nc.vector.scalar_tensor_tensor(
                    out=tmp2[:], in0=m2col[:], scalar=-(C0 - C1), in1=c0t[:],
                    op0=OP.mult, op1=OP.add)
                nc.vector.tensor_mul(rcat[:, 0:NT], tmp2[:], mval[:])
                nc.vector.tensor_mul(rcat[:, NT:2 * NT], logp0[:], epw[:])
                nc.vector.tensor_mul(rcat[:, 2 * NT:3 * NT], log1m[:], ehw[:])

                m1c = small.tile([64, BPC], F32, tag="m1c")
                nc.vector.tensor_scalar_max(m1c[:], m1all[:], EPS)
                m1c2 = small.tile([64, BPC], F32, tag="m1c2")
                nc.vector.tensor_scalar_min(m1c2[:], m1c[:], HI)
                lgm1 = small.tile([64, BPC], F32, tag="lgm1")
                nc.scalar.activation(lgm1[:], m1c2[:], AF.Ln)
                nc.vector.tensor_mul(t1col[:], lgm1[:], mvt[:])

                psA = finp.tile([1, 3 * NT], F32, tag="psA")
                nc.tensor.matmul(out=psA[:], lhsT=ones[:], rhs=rcat[:],
                                 start=True, stop=True)
                psB = finp.tile([1, BPC], F32, tag="psB")
                nc.tensor.matmul(out=psB[:], lhsT=ones[0:64, :], rhs=t1col[:],
                                 start=True, stop=True)

                a_t2 = small.tile([1, 1], F32, tag="a_t2")
                nc.vector.tensor_reduce(a_t2[:], psA[0:1, 0:NT],
                                        axis=mybir.AxisListType.X, op=OP.add)
                a_eos = small.tile([1, 1], F32, tag="a_eos")
                nc.vector.tensor_reduce(a_eos[:], psA[0:1, NT:3 * NT],
                                        axis=mybir.AxisListType.X, op=OP.add)
                a_t1 = small.tile([1, 1], F32, tag="a_t1")
                nc.vector.tensor_reduce(a_t1[:], psB[0:1, :],
                                        axis=mybir.AxisListType.X, op=OP.add)
                out_t = small.tile([1, 2], F32, tag="out_t")
                nc.vector.tensor_sub(out_t[:, 0:1], a_t2[:], a_t1[:])
                nc.vector.tensor_copy(out_t[:, 1:2], a_eos[:])
                nc.sync.dma_start(out_d[:], out_t[:])

    nc.compile()
    return nc


def _prep_core_inputs(logits, targets, core):
    """Host-side marshaling for one core (batches core*BPC .. core*BPC+BPC-1)."""
    b0 = core * BPC
    x = np.ascontiguousarray(
        logits[b0:b0 + BPC].reshape(ROWS, V), dtype=np.float32)
    tg = np.asarray(targets[b0:b0 + BPC])
    valid = (tg != 0) & (tg != PAD)                        # [BPC, T]
    tgc = np.where(valid, tg, 0).astype(np.int16)
    validf = valid.astype(np.float32)
    ep = (tg == 0).astype(np.float32)

    idx = np.zeros((P, 4 * NT), dtype=np.int16)
    mj = np.zeros((P, NJ * NT), dtype=np.float32)
    mval = np.zeros((P, NT), dtype=np.float32)
    epw = np.zeros((P, NT), dtype=np.float32)
    ehw = np.zeros((P, NT), dtype=np.float32)
    ep_w = -0.5 / (B * (ep.sum(axis=1) + EPS))             # [BPC]
    eh_w = -0.5 / (B * (validf.sum(axis=1) + EPS))
    for i in range(NT):
        for g in range(8):
            bl = 2 * i + g // 4
            for n in range(NJ):
                idx[16 * g + (n % 16), 4 * i + n // 16] = tgc[bl, n]
        p = np.arange(P)
        bl_of_p = 2 * i + p // 64
        mj[:, NJ * i:NJ * i + 64] = validf[bl_of_p, :]
        mval[:, i] = validf[bl_of_p, p % 64]
        epw[:, i] = ep[bl_of_p, p % 64] * ep_w[bl_of_p]
        ehw[:, i] = validf[bl_of_p, p % 64] * eh_w[bl_of_p]
    mvt = np.ascontiguousarray(validf.T)                   # [T=64, BPC]
    ident = np.zeros((P, 64), dtype=np.float32)
    ident[np.arange(P), np.arange(P) % 64] = 1.0
    ones = np.ones((P, 1), dtype=np.float32)
    return {"x": x, "idx": idx, "mj": mj, "mval": mval, "epw": epw,
            "ehw": ehw, "mvt": mvt, "ident": ident, "ones": ones}


_CACHE = {}


def _get_runner():
    """Build the Bass program and a cached 8-core PJRT executable."""
    if "runner" in _CACHE:
        return _CACHE["runner"]
    import jax
    from jax.sharding import Mesh, PartitionSpec
    from jax.experimental.shard_map import shard_map
    from concourse import bass2jax

    nc = _build_program()
    bass2jax.install_neuronx_cc_hook()

    part_name = nc.partition_id_tensor.name if nc.partition_id_tensor else None
    in_names, out_names, out_avals, zero_outs = [], [], [], []
    for alloc in nc.m.functions[0].allocations:
        if not isinstance(alloc, mybir.MemoryLocationSet):
            continue
        name = alloc.memorylocations[0].name
        if alloc.kind == "ExternalInput":
            if name != part_name:
                in_names.append(name)
        elif alloc.kind == "ExternalOutput":
            out_names.append(name)
            shape = tuple(alloc.tensor_shape)
            dtype = mybir.dt.np(alloc.dtype)
            out_avals.append(jax.core.ShapedArray(shape, dtype))
            zero_outs.append(np.zeros(shape, dtype))
    n_params = len(in_names)
    all_names = in_names + out_names
    if part_name is not None:
        all_names = all_names + [part_name]

    def _body(*args):
        operands = list(args)
        if part_name is not None:
            operands.append(bass2jax.partition_id_tensor())
        outs = bass2jax._bass_exec_p.bind(
            *operands,
            out_avals=tuple(out_avals),
            in_names=tuple(all_names),
            out_names=tuple(out_names),
            lowering_input_output_aliases=(),
            sim_require_finite=True,
            sim_require_nnan=True,
            nc=nc,
        )
        return tuple(outs)

    devices = jax.devices()[:N_CORES]
    mesh = Mesh(np.asarray(devices), ("core",))
    donate = tuple(range(n_params, n_params + len(out_names)))
    sharded = jax.jit(
        shard_map(_body, mesh=mesh,
                  in_specs=(PartitionSpec("core"),) * (n_params + len(out_names)),
                  out_specs=(PartitionSpec("core"),) * len(out_names),
                  check_rep=False),
        donate_argnums=donate, keep_unused=True)

    runner = (sharded, in_names, out_names, zero_outs)
    _CACHE["runner"] = runner
    return runner


def run_device(in_maps):
    """Run the SPMD program; in_maps is a list of N_CORES dicts."""
    sharded, in_names, out_names, zero_outs = _get_runner()
    concat_in = [
        np.concatenate([in_maps[c][n] for c in range(N_CORES)], axis=0)
        for n in in_names
    ]
    concat_zero = [
        np.zeros((N_CORES * z.shape[0], *z.shape[1:]), z.dtype) for z in zero_outs
    ]
    out_arrs = sharded(*concat_in, *concat_zero)
    out0 = np.asarray(out_arrs[0]).reshape(N_CORES, 1, 2)
    return out0


def kernel(logits, targets):
    logits = np.asarray(logits)
    targets = np.asarray(targets)
    in_maps = [_prep_core_inputs(logits, targets, c) for c in range(N_CORES)]
    outs = run_device(in_maps)                             # [N_CORES, 1, 2]
    label = outs[:, 0, 0].sum(dtype=np.float64)
    eos = outs[:, 0, 1].sum(dtype=np.float64)
    return (np.float32(label), np.float32(eos))



# revision 9
# speedup vs baseline: 1.6937x; 1.6937x over previous
"""Chamfer-distance criterion kernel for Trainium2 (8 NeuronCores, data-parallel over batch).

Math: the reference's two [B,T,T] pairwise cross-entropy GEMMs collapse exactly
because one side of each GEMM is a (masked) one-hot:

  probs = softmax(logits); p0 = probs[:,:,0]; valid = (t!=0)&(t!=PAD)
  knn_ce(one_hot, xs) = sum_{valid n} clamp(-amax1_n, C1, C0)
     with amax1_n = max_{valid m} (l[m, t_n] - logZ_m)
  knn_ce(xs, one_hot) = sum_n valid_n*(C0*(1-p0) - (C0-C1)*exp(gmax_n - logZ_n))
     with gmax_n = max_{valid j} l[n, t_j], C0 = -log(eps), C1 = -log1p(-(D-1)eps)

So each core needs, per row, only Z = sum(exp(l)) over the full vocab (the
fp16-streamed pass, ACT-engine bound) plus tiny reductions over the host-
gathered f32 logit columns at the <=64 target ids per batch.  No on-device
gather and no exp/log over the gathered matrix: max commutes with exp, and
-log(clip(exp(a))) == clamp(-a, C1, C0).
"""

import math
import os
import numpy as np
from contextlib import ExitStack

STAGE = int(os.environ.get("KSTAGE", "9"))  # debug bisect knob; 9 = full

import concourse.bass as bass
import concourse.tile as tile
from concourse import bacc, mybir

# ---- problem constants (hardcoded per contract) ----
B, T, V = 64, 64, 8192
PAD = 8192
EPS = 1e-8
D = V - 1
C0 = float(-math.log(EPS))
C1 = float(-math.log1p(-(D - 1) * EPS))
BIG = 50.0                 # additive log-domain mask; BIG > C0 + max|l| + max logZ

N_CORES = 8
BPC = B // N_CORES          # batches per core = 8
ROWS = BPC * T              # rows per core = 512
P = 128                     # partitions per tile
NT = ROWS // P              # tiles per core = 4
NJ = 64                     # gather slots: one per target position
F32 = mybir.dt.float32
F16 = mybir.dt.float16


def _build_program(reps=1):
    nc = bacc.Bacc("TRN2", target_bir_lowering=False, debug=False)
    x_d = nc.dram_tensor("x", [ROWS, V], F16, kind="ExternalInput").ap()
    g_d = nc.dram_tensor("g", [P, NT, NJ], F32, kind="ExternalInput").ap()
    gm_d = nc.dram_tensor("gm", [P, NT, NJ], F32, kind="ExternalInput").ap()
    hostm_d = nc.dram_tensor("hostm", [P, NT], F32, kind="ExternalInput").ap()
    l0_d = nc.dram_tensor("l0", [P, NT], F32, kind="ExternalInput").ap()
    mval_d = nc.dram_tensor("mval", [P, NT], F32, kind="ExternalInput").ap()
    epw_d = nc.dram_tensor("epw", [P, NT], F32, kind="ExternalInput").ap()
    ehw_d = nc.dram_tensor("ehw", [P, NT], F32, kind="ExternalInput").ap()
    mvt_d = nc.dram_tensor("mvt", [64, BPC], F32, kind="ExternalInput").ap()
    ident_d = nc.dram_tensor("ident", [P, 64], F32, kind="ExternalInput").ap()
    ones_d = nc.dram_tensor("ones", [P, 1], F32, kind="ExternalInput").ap()
    out_d = nc.dram_tensor("out", [1, 2], F32, kind="ExternalOutput").ap()

    AF = mybir.ActivationFunctionType
    OP = mybir.AluOpType

    with tile.TileContext(nc) as tc:
        with ExitStack() as ctx:
            const = ctx.enter_context(tc.tile_pool(name="const", bufs=1))
            xp = ctx.enter_context(tc.tile_pool(name="xp", bufs=3))
            epool = ctx.enter_context(tc.tile_pool(name="ep", bufs=2))
            apool = ctx.enter_context(tc.tile_pool(name="apool", bufs=2))
            small = ctx.enter_context(tc.tile_pool(name="small", bufs=1))
            tpp = ctx.enter_context(tc.tile_pool(name="tpp", bufs=4, space="PSUM"))
            finp = ctx.enter_context(tc.tile_pool(name="finp", bufs=1, space="PSUM"))

            # constants / marshaled inputs (DMA'd once; reused every rep)
            ident = const.tile([P, 64], F32)
            nc.sync.dma_start(ident[:], ident_d[:])
            ones = const.tile([P, 1], F32)
            nc.sync.dma_start(ones[:], ones_d[:])
            g3 = const.tile([P, NT, NJ], F32)
            nc.sync.dma_start(g3[:], g_d[:])
            gm3 = const.tile([P, NT, NJ], F32)
            nc.sync.dma_start(gm3[:], gm_d[:])
            hostm = const.tile([P, NT], F32)
            nc.sync.dma_start(hostm[:], hostm_d[:])
            l0 = const.tile([P, NT], F32)
            nc.sync.dma_start(l0[:], l0_d[:])
            mval = const.tile([P, NT], F32)
            nc.sync.dma_start(mval[:], mval_d[:])
            epw = const.tile([P, NT], F32)
            nc.sync.dma_start(epw[:], epw_d[:])
            ehw = const.tile([P, NT], F32)
            nc.sync.dma_start(ehw[:], ehw_d[:])
            mvt = const.tile([64, BPC], F32)
            nc.sync.dma_start(mvt[:], mvt_d[:])

            for rep in range(reps):
                # gmax over valid targets: no dependency on Z -> runs during stream
                gmx = small.tile([P, NT], F32, tag="gmx")
                if STAGE >= 4:
                    nc.vector.tensor_reduce(gmx[:], gm3[:],
                                            axis=mybir.AxisListType.X, op=OP.max)
                else:
                    nc.vector.tensor_copy(gmx[:], hostm[:])

                # ---- streaming pass: exp + row-sum per [128, V] fp16 tile ----
                zcol = small.tile([P, NT], F32, tag="zcol")
                for i in range(NT):
                    xt = xp.tile([P, V], F16, tag="xt")
                    nc.sync.dma_start(xt[:], x_d[i * P:(i + 1) * P, :])
                    et = epool.tile([P, V], F16, tag="et")
                    nc.scalar.activation(et[:], xt[:], AF.Exp,
                                         accum_out=zcol[:, i:i + 1])
                if STAGE <= 1:
                    out_t = small.tile([1, 2], F32, tag="out_t")
                    nc.vector.tensor_copy(out_t[:], zcol[0:1, 0:2])
                    nc.sync.dma_start(out_d[:], out_t[:])
                    continue

                # ---- epilogue ----
                lz = small.tile([P, NT], F32, tag="lz")
                nc.scalar.activation(lz[:], zcol[:], AF.Ln)
                # lzm = -logZ - BIG*invalid_row   (hostm = 0 / -BIG)
                lzm = small.tile([P, NT], F32, tag="lzm")
                nc.vector.scalar_tensor_tensor(
                    out=lzm[:], in0=lz[:], scalar=-1.0, in1=hostm[:],
                    op0=OP.mult, op1=OP.add)
                ecat = small.tile([P, 2 * NT], F32, tag="ecat")
                nc.vector.tensor_add(ecat[:, 0:NT], gmx[:], lzm[:])
                # logp0 = l0 - logZ (unmasked)
                nc.vector.scalar_tensor_tensor(
                    out=ecat[:, NT:2 * NT], in0=lz[:], scalar=-1.0, in1=l0[:],
                    op0=OP.mult, op1=OP.add)
                if STAGE <= 2:
                    out_t = small.tile([1, 2], F32, tag="out_t")
                    nc.vector.tensor_copy(out_t[:], ecat[0:1, 0:2])
                    nc.sync.dma_start(out_d[:], out_t[:])
                    continue
                pcat = small.tile([P, 2 * NT], F32, tag="pcat")
                nc.scalar.activation(pcat[:], ecat[:], AF.Exp)  # [m2 | p0]
                onem = small.tile([P, NT], F32, tag="onem")
                nc.vector.tensor_scalar(
                    out=onem[:], in0=pcat[:, NT:2 * NT], scalar1=-1.0,
                    scalar2=1.0, op0=OP.mult, op1=OP.add)       # 1 - p0
                log1m = small.tile([P, NT], F32, tag="log1m")
                nc.scalar.activation(log1m[:], onem[:], AF.Ln)
                c0t = small.tile([P, NT], F32, tag="c0t")
                nc.vector.tensor_scalar_mul(c0t[:], onem[:], C0)  # C0*(1-p0)
                tmp2 = small.tile([P, NT], F32, tag="tmp2")
                # tmp2 = C0*(1-p0) - (C0-C1)*m2
                nc.vector.scalar_tensor_tensor(
                    out=tmp2[:], in0=pcat[:, 0:NT], scalar=-(C0 - C1),
                    in1=c0t[:], op0=OP.mult, op1=OP.add)
                rcat = small.tile([P, 3 * NT], F32, tag="rcat")
                nc.vector.tensor_mul(rcat[:, 0:NT], tmp2[:], mval[:])
                nc.vector.tensor_mul(rcat[:, NT:2 * NT], ecat[:, NT:2 * NT], epw[:])
                nc.vector.tensor_mul(rcat[:, 2 * NT:3 * NT], log1m[:], ehw[:])

                # ---- term1: cross-row max of a = g - logZ_row - BIG*invalid_row
                m1all = small.tile([64, BPC], F32, tag="m1all")
                if STAGE >= 5:
                    for i in range(NT):
                        at = apool.tile([P, NJ], F32, tag="at")
                        nc.vector.tensor_scalar_add(at[:], g3[:, i, :],
                                                    scalar1=lzm[:, i:i + 1])
                        for h in range(2):
                            b = 2 * i + h
                            tp = tpp.tile([64, 64], F32, tag="tp")
                            nc.tensor.transpose(tp[:],
                                                at[64 * h:64 * h + 64, :],
                                                ident[64 * h:64 * h + 64, :])
                            nc.vector.tensor_reduce(m1all[:, b:b + 1], tp[:],
                                                    axis=mybir.AxisListType.X,
                                                    op=OP.max)
                else:
                    nc.vector.tensor_copy(m1all[:], mvt[:])
                # t1 = clamp(-amax1, C1, C0) * valid_n
                t1a = small.tile([64, BPC], F32, tag="t1a")
                nc.vector.tensor_scalar(
                    out=t1a[:], in0=m1all[:], scalar1=-1.0, scalar2=C1,
                    op0=OP.mult, op1=OP.max)
                t1b = small.tile([64, BPC], F32, tag="t1b")
                nc.vector.tensor_scalar_min(t1b[:], t1a[:], C0)
                t1col = small.tile([64, BPC], F32, tag="t1col")
                nc.vector.tensor_mul(t1col[:], t1b[:], mvt[:])

                # ---- final partition-dim sums via matmul with ones ----
                psA = finp.tile([1, 3 * NT], F32, tag="psA")
                nc.tensor.matmul(out=psA[:], lhsT=ones[:], rhs=rcat[:],
                                 start=True, stop=True)
                psB = finp.tile([1, BPC], F32, tag="psB")
                nc.tensor.matmul(out=psB[:], lhsT=ones[0:64, :], rhs=t1col[:],
                                 start=True, stop=True)

                a_t2 = small.tile([1, 1], F32, tag="a_t2")
                nc.vector.tensor_reduce(a_t2[:], psA[0:1, 0:NT],
                                        axis=mybir.AxisListType.X, op=OP.add)
                a_eos = small.tile([1, 1], F32, tag="a_eos")
                nc.vector.tensor_reduce(a_eos[:], psA[0:1, NT:3 * NT],
                                        axis=mybir.AxisListType.X, op=OP.add)
                a_t1 = small.tile([1, 1], F32, tag="a_t1")
                nc.vector.tensor_reduce(a_t1[:], psB[0:1, :],
                                        axis=mybir.AxisListType.X, op=OP.add)
                out_t = small.tile([1, 2], F32, tag="out_t")
                nc.vector.tensor_add(out_t[:, 0:1], a_t2[:], a_t1[:])
                nc.vector.tensor_copy(out_t[:, 1:2], a_eos[:])
                nc.sync.dma_start(out_d[:], out_t[:])

    nc.compile()
    return nc


def _prep_core_inputs(logits, targets, core):
    """Host-side marshaling for one core (batches core*BPC .. core*BPC+BPC-1)."""
    b0 = core * BPC
    lg = np.asarray(logits[b0:b0 + BPC], dtype=np.float32)  # [BPC, T, V]
    x = np.ascontiguousarray(lg.reshape(ROWS, V).astype(np.float16))
    tg = np.asarray(targets[b0:b0 + BPC])
    valid = (tg != 0) & (tg != PAD)                         # [BPC, T]
    tgc = np.where(valid, tg, 0).astype(np.int64)
    validf = valid.astype(np.float32)
    ep = (tg == 0).astype(np.float32)
    ep_w = -0.5 / (B * (ep.sum(axis=1) + EPS))              # [BPC]
    eh_w = -0.5 / (B * (validf.sum(axis=1) + EPS))

    g = np.zeros((P, NT, NJ), dtype=np.float32)
    gm = np.zeros((P, NT, NJ), dtype=np.float32)
    hostm = np.zeros((P, NT), dtype=np.float32)
    l0 = np.zeros((P, NT), dtype=np.float32)
    mval = np.zeros((P, NT), dtype=np.float32)
    epw = np.zeros((P, NT), dtype=np.float32)
    ehw = np.zeros((P, NT), dtype=np.float32)
    p = np.arange(P)
    r = p % 64
    for i in range(NT):
        bl = 2 * i + p // 64                                # [P]
        g[:, i, :] = lg[bl[:, None], r[:, None], tgc[bl, :]]
        gm[:, i, :] = g[:, i, :] + (validf[bl, :] - 1.0) * BIG
        hostm[:, i] = (validf[bl, r] - 1.0) * BIG
        l0[:, i] = lg[bl, r, 0]
        mval[:, i] = validf[bl, r]
        epw[:, i] = ep[bl, r] * ep_w[bl]
        ehw[:, i] = validf[bl, r] * eh_w[bl]
    mvt = np.ascontiguousarray(validf.T)                    # [T=64, BPC]
    ident = np.zeros((P, 64), dtype=np.float32)
    ident[np.arange(P), np.arange(P) % 64] = 1.0
    ones = np.ones((P, 1), dtype=np.float32)
    return {"x": x, "g": g, "gm": gm, "hostm": hostm, "l0": l0, "mval": mval,
            "epw": epw, "ehw": ehw, "mvt": mvt, "ident": ident, "ones": ones}


_CACHE = {}


def _get_runner():
    """Build the Bass program and a cached 8-core PJRT executable."""
    if "runner" in _CACHE:
        return _CACHE["runner"]
    import jax
    from jax.sharding import Mesh, PartitionSpec
    from jax.experimental.shard_map import shard_map
    from concourse import bass2jax

    nc = _build_program()
    bass2jax.install_neuronx_cc_hook()

    part_name = nc.partition_id_tensor.name if nc.partition_id_tensor else None
    in_names, out_names, out_avals, zero_outs = [], [], [], []
    for alloc in nc.m.functions[0].allocations:
        if not isinstance(alloc, mybir.MemoryLocationSet):
            continue
        name = alloc.memorylocations[0].name
        if alloc.kind == "ExternalInput":
            if name != part_name:
                in_names.append(name)
        elif alloc.kind == "ExternalOutput":
            out_names.append(name)
            shape = tuple(alloc.tensor_shape)
            dtype = mybir.dt.np(alloc.dtype)
            out_avals.append(jax.core.ShapedArray(shape, dtype))
            zero_outs.append(np.zeros(shape, dtype))
    n_params = len(in_names)
    all_names = in_names + out_names
    if part_name is not None:
        all_names = all_names + [part_name]

    def _body(*args):
        operands = list(args)
        if part_name is not None:
            operands.append(bass2jax.partition_id_tensor())
        outs = bass2jax._bass_exec_p.bind(
            *operands,
            out_avals=tuple(out_avals),
            in_names=tuple(all_names),
            out_names=tuple(out_names),
            lowering_input_output_aliases=(),
            sim_require_finite=True,
            sim_require_nnan=True,
            nc=nc,
        )
        return tuple(outs)

    devices = jax.devices()[:N_CORES]
    mesh = Mesh(np.asarray(devices), ("core",))
    donate = tuple(range(n_params, n_params + len(out_names)))
    sharded = jax.jit(
        shard_map(_body, mesh=mesh,
                  in_specs=(PartitionSpec("core"),) * (n_params + len(out_names)),
                  out_specs=(PartitionSpec("core"),) * len(out_names),
                  check_rep=False),
        donate_argnums=donate, keep_unused=True)

    runner = (sharded, in_names, out_names, zero_outs)
    _CACHE["runner"] = runner
    return runner


def run_device(in_maps):
    """Run the SPMD program; in_maps is a list of N_CORES dicts."""
    sharded, in_names, out_names, zero_outs = _get_runner()
    concat_in = [
        np.concatenate([in_maps[c][n] for c in range(N_CORES)], axis=0)
        for n in in_names
    ]
    concat_zero = [
        np.zeros((N_CORES * z.shape[0], *z.shape[1:]), z.dtype) for z in zero_outs
    ]
    out_arrs = sharded(*concat_in, *concat_zero)
    out0 = np.asarray(out_arrs[0]).reshape(N_CORES, 1, 2)
    return out0


def kernel(logits, targets):
    logits = np.asarray(logits)
    targets = np.asarray(targets)
    in_maps = [_prep_core_inputs(logits, targets, c) for c in range(N_CORES)]
    outs = run_device(in_maps)                             # [N_CORES, 1, 2]
    label = outs[:, 0, 0].sum(dtype=np.float64)
    eos = outs[:, 0, 1].sum(dtype=np.float64)
    return (np.float32(label), np.float32(eos))


# revision 10
# speedup vs baseline: 3.0862x; 1.8221x over previous
"""Chamfer-distance criterion kernel for Trainium2 (8 NeuronCores, data-parallel over batch).

Math: the reference's two [B,T,T] pairwise cross-entropy GEMMs collapse exactly
because one side of each GEMM is a (masked) one-hot:

  probs = softmax(logits); p0 = probs[:,:,0]; valid = (t!=0)&(t!=PAD)
  knn_ce(one_hot, xs) = sum_{valid n} clamp(-amax1_n, C1, C0)
     with amax1_n = max_{valid m} (l[m, t_n] - logZ_m)
  knn_ce(xs, one_hot) = sum_n valid_n*(C0*(1-p0) - (C0-C1)*exp(gmax_n - logZ_n))
     with gmax_n = max_{valid j} l[n, t_j], C0 = -log(eps), C1 = -log1p(-(D-1)eps)

So each core needs, per row, only Z = sum(exp(l)) over the full vocab, plus
tiny reductions over host-gathered f32 logit columns at the <=64 target ids
per batch (max commutes with exp; -log(clip(exp(a))) == clamp(-a, C1, C0)).

The Z pass streams the logits as fp8_e4m3 (host cast; ~0.2% rms Z error, far
inside the 2e-2 gate) and splits the vocab between two engines per row-tile:
  cols [0, VA):   ACT exp (double-rate at fp8) + hardware row-accumulate
  cols [VA, V):   DVE Schraudolph exp -- i32(x*2^23/ln2 + B) bitcast to f32
                  IS ~exp(x); one tensor_scalar convert + one accum-sum
"""

import math
import os
import numpy as np
from contextlib import ExitStack

import concourse.bass as bass
import concourse.tile as tile
from concourse import bacc, mybir

# ---- problem constants (hardcoded per contract) ----
B, T, V = 64, 64, 8192
PAD = 8192
EPS = 1e-8
D = V - 1
C0 = float(-math.log(EPS))
C1 = float(-math.log1p(-(D - 1) * EPS))
BIG = 50.0                 # additive log-domain mask; BIG > C0 + max|l| + max logZ

N_CORES = 8
BPC = B // N_CORES          # batches per core = 8
ROWS = BPC * T              # rows per core = 512
P = 128                     # partitions per tile
NT = ROWS // P              # row tiles per core = 4
NJ = 64                     # gather slots: one per target position
VA = int(os.environ.get("KVA", "6016"))   # ACT's column share (mult of 64)
VB = V - VA                               # DVE's column share
SCH_S = float(2.0 ** 23 / math.log(2.0))  # Schraudolph scale
SCH_B = float(127.0 * 2 ** 23 - 486411.0)  # Schraudolph bias (mean-centered)
F32 = mybir.dt.float32
F16 = mybir.dt.float16
F8 = mybir.dt.float8e4
I32 = mybir.dt.int32


def _build_program(reps=1):
    nc = bacc.Bacc("TRN2", target_bir_lowering=False, debug=False)
    x_d = nc.dram_tensor("x", [P, NT * V], F8, kind="ExternalInput").ap()
    g_d = nc.dram_tensor("g", [P, NT, NJ], F32, kind="ExternalInput").ap()
    gm_d = nc.dram_tensor("gm", [P, NT, NJ], F32, kind="ExternalInput").ap()
    hostm_d = nc.dram_tensor("hostm", [P, NT], F32, kind="ExternalInput").ap()
    l0_d = nc.dram_tensor("l0", [P, NT], F32, kind="ExternalInput").ap()
    mval_d = nc.dram_tensor("mval", [P, NT], F32, kind="ExternalInput").ap()
    epw_d = nc.dram_tensor("epw", [P, NT], F32, kind="ExternalInput").ap()
    ehw_d = nc.dram_tensor("ehw", [P, NT], F32, kind="ExternalInput").ap()
    mvt_d = nc.dram_tensor("mvt", [64, BPC], F32, kind="ExternalInput").ap()
    ident_d = nc.dram_tensor("ident", [P, 64], F32, kind="ExternalInput").ap()
    ones_d = nc.dram_tensor("ones", [P, 1], F32, kind="ExternalInput").ap()
    out_d = nc.dram_tensor("out", [1, 2], F32, kind="ExternalOutput").ap()

    AF = mybir.ActivationFunctionType
    OP = mybir.AluOpType

    with tile.TileContext(nc) as tc:
        with ExitStack() as ctx:
            const = ctx.enter_context(tc.tile_pool(name="const", bufs=1))
            xp = ctx.enter_context(tc.tile_pool(name="xp", bufs=2))
            epool = ctx.enter_context(tc.tile_pool(name="ep", bufs=2))
            ipool = ctx.enter_context(tc.tile_pool(name="ip", bufs=2))
            spool = ctx.enter_context(tc.tile_pool(name="sp", bufs=2))
            apool = ctx.enter_context(tc.tile_pool(name="apool", bufs=2))
            small = ctx.enter_context(tc.tile_pool(name="small", bufs=1))
            tpp = ctx.enter_context(tc.tile_pool(name="tpp", bufs=4, space="PSUM"))
            finp = ctx.enter_context(tc.tile_pool(name="finp", bufs=1, space="PSUM"))

            # constants / marshaled inputs (DMA'd once; reused every rep)
            ident = const.tile([P, 64], F32)
            nc.sync.dma_start(ident[:], ident_d[:])
            ones = const.tile([P, 1], F32)
            nc.sync.dma_start(ones[:], ones_d[:])
            g3 = const.tile([P, NT, NJ], F32)
            nc.sync.dma_start(g3[:], g_d[:])
            gm3 = const.tile([P, NT, NJ], F32)
            nc.sync.dma_start(gm3[:], gm_d[:])
            hostm = const.tile([P, NT], F32)
            nc.sync.dma_start(hostm[:], hostm_d[:])
            l0 = const.tile([P, NT], F32)
            nc.sync.dma_start(l0[:], l0_d[:])
            mval = const.tile([P, NT], F32)
            nc.sync.dma_start(mval[:], mval_d[:])
            epw = const.tile([P, NT], F32)
            nc.sync.dma_start(epw[:], epw_d[:])
            ehw = const.tile([P, NT], F32)
            nc.sync.dma_start(ehw[:], ehw_d[:])
            mvt = const.tile([64, BPC], F32)
            nc.sync.dma_start(mvt[:], mvt_d[:])

            for rep in range(reps):
                # gmax over valid targets: no dependency on Z -> runs during stream
                gmx = small.tile([P, NT], F32, tag="gmx")
                nc.vector.tensor_reduce(gmx[:], gm3[:],
                                        axis=mybir.AxisListType.X, op=OP.max)

                # ---- streaming Z pass over the fp8 logits ----
                xt4 = xp.tile([P, NT, V], F8, tag="xt")
                nc.sync.dma_start(xt4[:], x_d[:])
                zA = small.tile([P, NT], F32, tag="zA")
                zB = small.tile([P, NT], F32, tag="zB")
                for i in range(NT):
                    eta = epool.tile([P, VA], F16, tag="eta")
                    nc.scalar.activation(eta[:], xt4[:, i, 0:VA], AF.Exp,
                                         accum_out=zA[:, i:i + 1])
                    it32 = ipool.tile([P, VB], I32, tag="it")
                    nc.vector.tensor_scalar(
                        out=it32[:], in0=xt4[:, i, VA:V], scalar1=SCH_S,
                        scalar2=SCH_B, op0=OP.mult, op1=OP.add)
                    st = spool.tile([P, VB], F32, tag="st")
                    nc.vector.tensor_scalar(
                        out=st[:], in0=it32[:].bitcast(F32), scalar1=0.0,
                        scalar2=None, op0=OP.add, op1=OP.add,
                        accum_out=zB[:, i:i + 1])
                zcol = small.tile([P, NT], F32, tag="zcol")
                nc.vector.tensor_add(zcol[:], zA[:], zB[:])

                # ---- epilogue ----
                lz = small.tile([P, NT], F32, tag="lz")
                nc.scalar.activation(lz[:], zcol[:], AF.Ln)
                # lzm = -logZ - BIG*invalid_row   (hostm = 0 / -BIG)
                lzm = small.tile([P, NT], F32, tag="lzm")
                nc.vector.scalar_tensor_tensor(
                    out=lzm[:], in0=lz[:], scalar=-1.0, in1=hostm[:],
                    op0=OP.mult, op1=OP.add)
                ecat = small.tile([P, 2 * NT], F32, tag="ecat")
                nc.vector.tensor_add(ecat[:, 0:NT], gmx[:], lzm[:])
                # logp0 = l0 - logZ (unmasked)
                nc.vector.scalar_tensor_tensor(
                    out=ecat[:, NT:2 * NT], in0=lz[:], scalar=-1.0, in1=l0[:],
                    op0=OP.mult, op1=OP.add)
                pcat = small.tile([P, 2 * NT], F32, tag="pcat")
                nc.scalar.activation(pcat[:], ecat[:], AF.Exp)  # [m2 | p0]
                onem = small.tile([P, NT], F32, tag="onem")
                nc.vector.tensor_scalar(
                    out=onem[:], in0=pcat[:, NT:2 * NT], scalar1=-1.0,
                    scalar2=1.0, op0=OP.mult, op1=OP.add)       # 1 - p0
                log1m = small.tile([P, NT], F32, tag="log1m")
                nc.scalar.activation(log1m[:], onem[:], AF.Ln)
                c0t = small.tile([P, NT], F32, tag="c0t")
                nc.vector.tensor_scalar_mul(c0t[:], onem[:], C0)  # C0*(1-p0)
                tmp2 = small.tile([P, NT], F32, tag="tmp2")
                # tmp2 = C0*(1-p0) - (C0-C1)*m2
                nc.vector.scalar_tensor_tensor(
                    out=tmp2[:], in0=pcat[:, 0:NT], scalar=-(C0 - C1),
                    in1=c0t[:], op0=OP.mult, op1=OP.add)
                rcat = small.tile([P, 3 * NT], F32, tag="rcat")
                nc.vector.tensor_mul(rcat[:, 0:NT], tmp2[:], mval[:])
                nc.vector.tensor_mul(rcat[:, NT:2 * NT], ecat[:, NT:2 * NT], epw[:])
                nc.vector.tensor_mul(rcat[:, 2 * NT:3 * NT], log1m[:], ehw[:])

                # ---- term1: cross-row max of a = g - logZ_row - BIG*invalid_row
                m1all = small.tile([64, BPC], F32, tag="m1all")
                for i in range(NT):
                    at = apool.tile([P, NJ], F32, tag="at")
                    nc.vector.tensor_scalar_add(at[:], g3[:, i, :],
                                                scalar1=lzm[:, i:i + 1])
                    for h in range(2):
                        b = 2 * i + h
                        tp = tpp.tile([64, 64], F32, tag="tp")
                        nc.tensor.transpose(tp[:],
                                            at[64 * h:64 * h + 64, :],
                                            ident[64 * h:64 * h + 64, :])
                        nc.vector.tensor_reduce(m1all[:, b:b + 1], tp[:],
                                                axis=mybir.AxisListType.X,
                                                op=OP.max)
                # t1 = clamp(-amax1, C1, C0) * valid_n
                t1a = small.tile([64, BPC], F32, tag="t1a")
                nc.vector.tensor_scalar(
                    out=t1a[:], in0=m1all[:], scalar1=-1.0, scalar2=C1,
                    op0=OP.mult, op1=OP.max)
                t1b = small.tile([64, BPC], F32, tag="t1b")
                nc.vector.tensor_scalar_min(t1b[:], t1a[:], C0)
                t1col = small.tile([64, BPC], F32, tag="t1col")
                nc.vector.tensor_mul(t1col[:], t1b[:], mvt[:])

                # ---- final partition-dim sums via matmul with ones ----
                psA = finp.tile([1, 3 * NT], F32, tag="psA")
                nc.tensor.matmul(out=psA[:], lhsT=ones[:], rhs=rcat[:],
                                 start=True, stop=True)
                psB = finp.tile([1, BPC], F32, tag="psB")
                nc.tensor.matmul(out=psB[:], lhsT=ones[0:64, :], rhs=t1col[:],
                                 start=True, stop=True)

                a_t2 = small.tile([1, 1], F32, tag="a_t2")
                nc.vector.tensor_reduce(a_t2[:], psA[0:1, 0:NT],
                                        axis=mybir.AxisListType.X, op=OP.add)
                a_eos = small.tile([1, 1], F32, tag="a_eos")
                nc.vector.tensor_reduce(a_eos[:], psA[0:1, NT:3 * NT],
                                        axis=mybir.AxisListType.X, op=OP.add)
                a_t1 = small.tile([1, 1], F32, tag="a_t1")
                nc.vector.tensor_reduce(a_t1[:], psB[0:1, :],
                                        axis=mybir.AxisListType.X, op=OP.add)
                out_t = small.tile([1, 2], F32, tag="out_t")
                nc.vector.tensor_add(out_t[:, 0:1], a_t2[:], a_t1[:])
                nc.vector.tensor_copy(out_t[:, 1:2], a_eos[:])
                nc.sync.dma_start(out_d[:], out_t[:])

    nc.compile()
    return nc


def _prep_core_inputs(logits, targets, core):
    """Host-side marshaling for one core (batches core*BPC .. core*BPC+BPC-1)."""
    import ml_dtypes
    b0 = core * BPC
    lg = np.asarray(logits[b0:b0 + BPC], dtype=np.float32)  # [BPC, T, V]
    # [P, NT*V] fp8: x[p, i*V + v] = logit of row i*128+p, col v
    x = np.ascontiguousarray(
        lg.reshape(NT, P, V).transpose(1, 0, 2).reshape(P, NT * V)
    ).astype(ml_dtypes.float8_e4m3)
    tg = np.asarray(targets[b0:b0 + BPC])
    valid = (tg != 0) & (tg != PAD)                         # [BPC, T]
    tgc = np.where(valid, tg, 0).astype(np.int64)
    validf = valid.astype(np.float32)
    ep = (tg == 0).astype(np.float32)
    ep_w = -0.5 / (B * (ep.sum(axis=1) + EPS))              # [BPC]
    eh_w = -0.5 / (B * (validf.sum(axis=1) + EPS))

    g = np.zeros((P, NT, NJ), dtype=np.float32)
    gm = np.zeros((P, NT, NJ), dtype=np.float32)
    hostm = np.zeros((P, NT), dtype=np.float32)
    l0 = np.zeros((P, NT), dtype=np.float32)
    mval = np.zeros((P, NT), dtype=np.float32)
    epw = np.zeros((P, NT), dtype=np.float32)
    ehw = np.zeros((P, NT), dtype=np.float32)
    p = np.arange(P)
    r = p % 64
    for i in range(NT):
        bl = 2 * i + p // 64                                # [P]
        g[:, i, :] = lg[bl[:, None], r[:, None], tgc[bl, :]]
        gm[:, i, :] = g[:, i, :] + (validf[bl, :] - 1.0) * BIG
        hostm[:, i] = (validf[bl, r] - 1.0) * BIG
        l0[:, i] = lg[bl, r, 0]
        mval[:, i] = validf[bl, r]
        epw[:, i] = ep[bl, r] * ep_w[bl]
        ehw[:, i] = validf[bl, r] * eh_w[bl]
    mvt = np.ascontiguousarray(validf.T)                    # [T=64, BPC]
    ident = np.zeros((P, 64), dtype=np.float32)
    ident[np.arange(P), np.arange(P) % 64] = 1.0
    ones = np.ones((P, 1), dtype=np.float32)
    return {"x": x, "g": g, "gm": gm, "hostm": hostm, "l0": l0, "mval": mval,
            "epw": epw, "ehw": ehw, "mvt": mvt, "ident": ident, "ones": ones}


_CACHE = {}


def _get_runner():
    """Build the Bass program and a cached 8-core PJRT executable."""
    if "runner" in _CACHE:
        return _CACHE["runner"]
    import jax
    from jax.sharding import Mesh, PartitionSpec
    from jax.experimental.shard_map import shard_map
    from concourse import bass2jax

    nc = _build_program()
    bass2jax.install_neuronx_cc_hook()

    part_name = nc.partition_id_tensor.name if nc.partition_id_tensor else None
    in_names, out_names, out_avals, zero_outs = [], [], [], []
    for alloc in nc.m.functions[0].allocations:
        if not isinstance(alloc, mybir.MemoryLocationSet):
            continue
        name = alloc.memorylocations[0].name
        if alloc.kind == "ExternalInput":
            if name != part_name:
                in_names.append(name)
        elif alloc.kind == "ExternalOutput":
            out_names.append(name)
            shape = tuple(alloc.tensor_shape)
            dtype = mybir.dt.np(alloc.dtype)
            out_avals.append(jax.core.ShapedArray(shape, dtype))
            zero_outs.append(np.zeros(shape, dtype))
    n_params = len(in_names)
    all_names = in_names + out_names
    if part_name is not None:
        all_names = all_names + [part_name]

    def _body(*args):
        operands = list(args)
        if part_name is not None:
            operands.append(bass2jax.partition_id_tensor())
        outs = bass2jax._bass_exec_p.bind(
            *operands,
            out_avals=tuple(out_avals),
            in_names=tuple(all_names),
            out_names=tuple(out_names),
            lowering_input_output_aliases=(),
            sim_require_finite=True,
            sim_require_nnan=True,
            nc=nc,
        )
        return tuple(outs)

    devices = jax.devices()[:N_CORES]
    mesh = Mesh(np.asarray(devices), ("core",))
    donate = tuple(range(n_params, n_params + len(out_names)))
    sharded = jax.jit(
        shard_map(_body, mesh=mesh,
                  in_specs=(PartitionSpec("core"),) * (n_params + len(out_names)),
                  out_specs=(PartitionSpec("core"),) * len(out_names),
                  check_rep=False),
        donate_argnums=donate, keep_unused=True)

    runner = (sharded, in_names, out_names, zero_outs)
    _CACHE["runner"] = runner
    return runner


def run_device(in_maps):
    """Run the SPMD program; in_maps is a list of N_CORES dicts."""
    sharded, in_names, out_names, zero_outs = _get_runner()
    concat_in = [
        np.concatenate([in_maps[c][n] for c in range(N_CORES)], axis=0)
        for n in in_names
    ]
    concat_zero = [
        np.zeros((N_CORES * z.shape[0], *z.shape[1:]), z.dtype) for z in zero_outs
    ]
    out_arrs = sharded(*concat_in, *concat_zero)
    out0 = np.asarray(out_arrs[0]).reshape(N_CORES, 1, 2)
    return out0


def kernel(logits, targets):
    logits = np.asarray(logits)
    targets = np.asarray(targets)
    in_maps = [_prep_core_inputs(logits, targets, c) for c in range(N_CORES)]
    outs = run_device(in_maps)                             # [N_CORES, 1, 2]
    label = outs[:, 0, 0].sum(dtype=np.float64)
    eos = outs[:, 0, 1].sum(dtype=np.float64)
    return (np.float32(label), np.float32(eos))


# revision 15
# speedup vs baseline: 3.1220x; 1.0116x over previous
"""Chamfer-distance criterion kernel for Trainium2 (8 NeuronCores, data-parallel over batch).

Math: the reference's two [B,T,T] pairwise cross-entropy GEMMs collapse exactly
because one side of each GEMM is a (masked) one-hot:

  probs = softmax(logits); p0 = probs[:,:,0]; valid = (t!=0)&(t!=PAD)
  knn_ce(one_hot, xs) = sum_{valid n} clamp(-amax1_n, C1, C0)
     with amax1_n = max_{valid m} (l[m, t_n] - logZ_m)
  knn_ce(xs, one_hot) = sum_n valid_n*(C0*(1-p0) - (C0-C1)*exp(gmax_n - logZ_n))
     with gmax_n = max_{valid j} l[n, t_j], C0 = -log(eps), C1 = -log1p(-(D-1)eps)

So each core needs, per row, only Z = sum(exp(l)) over the full vocab, plus
tiny reductions over host-gathered f32 logit columns at the <=64 target ids
per batch (max commutes with exp; -log(clip(exp(a))) == clamp(-a, C1, C0)).

The Z pass streams the logits as fp8_e4m3 (host cast; ~0.2% rms Z error, far
inside the 2e-2 gate) and splits the vocab between two engines per row-tile:
  cols [0, VA):   ACT exp (double-rate at fp8) + hardware row-accumulate
  cols [VA, V):   DVE Schraudolph exp -- i32(x*2^23/ln2 + B) bitcast to f32
                  IS ~exp(x); one tensor_scalar convert + one accum-sum
"""

import math
import os
import numpy as np
from contextlib import ExitStack

import concourse.bass as bass
import concourse.tile as tile
from concourse import bacc, mybir

# ---- problem constants (hardcoded per contract) ----
B, T, V = 64, 64, 8192
PAD = 8192
EPS = 1e-8
D = V - 1
C0 = float(-math.log(EPS))
C1 = float(-math.log1p(-(D - 1) * EPS))
BIG = 50.0                 # additive log-domain mask; BIG > C0 + max|l| + max logZ

N_CORES = 8
BPC = B // N_CORES          # batches per core = 8
ROWS = BPC * T              # rows per core = 512
P = 128                     # partitions per tile
NT = ROWS // P              # row tiles per core = 4
NJ = 64                     # gather slots: one per target position
VA = int(os.environ.get("KVA", "6016"))   # ACT's column share (mult of 64)
VB = V - VA                               # DVE's column share
SCH_S = float(2.0 ** 23 / math.log(2.0))  # Schraudolph scale
SCH_B = float(127.0 * 2 ** 23 - 486411.0)  # Schraudolph bias (mean-centered)
F32 = mybir.dt.float32
F16 = mybir.dt.float16
F8 = mybir.dt.float8e4
I32 = mybir.dt.int32


def _build_program(reps=1):
    nc = bacc.Bacc("TRN2", target_bir_lowering=False, debug=False)
    x_d = nc.dram_tensor("x", [P, NT * V], F8, kind="ExternalInput").ap()
    g_d = nc.dram_tensor("g", [P, NT, NJ], F32, kind="ExternalInput").ap()
    gm_d = nc.dram_tensor("gm", [P, NT, NJ], F32, kind="ExternalInput").ap()
    hostm_d = nc.dram_tensor("hostm", [P, NT], F32, kind="ExternalInput").ap()
    l0_d = nc.dram_tensor("l0", [P, NT], F32, kind="ExternalInput").ap()
    mval_d = nc.dram_tensor("mval", [P, NT], F32, kind="ExternalInput").ap()
    epw_d = nc.dram_tensor("epw", [P, NT], F32, kind="ExternalInput").ap()
    ehw_d = nc.dram_tensor("ehw", [P, NT], F32, kind="ExternalInput").ap()
    mvt_d = nc.dram_tensor("mvt", [64, BPC], F32, kind="ExternalInput").ap()
    wcat_d = nc.dram_tensor("wcat", [P, 2 * NT], F32, kind="ExternalInput").ap()
    ident_d = nc.dram_tensor("ident", [P, 64], F32, kind="ExternalInput").ap()
    ones_d = nc.dram_tensor("ones", [P, 1], F32, kind="ExternalInput").ap()
    out_d = nc.dram_tensor("out", [1, 2], F32, kind="ExternalOutput").ap()

    AF = mybir.ActivationFunctionType
    OP = mybir.AluOpType

    with tile.TileContext(nc) as tc:
        with ExitStack() as ctx:
            const = ctx.enter_context(tc.tile_pool(name="const", bufs=1))
            xp = ctx.enter_context(tc.tile_pool(name="xp", bufs=2))
            epool = ctx.enter_context(tc.tile_pool(name="ep", bufs=2))
            ipool = ctx.enter_context(tc.tile_pool(name="ip", bufs=2))
            spool = ctx.enter_context(tc.tile_pool(name="sp", bufs=2))
            apool = ctx.enter_context(tc.tile_pool(name="apool", bufs=2))
            small = ctx.enter_context(tc.tile_pool(name="small", bufs=2))
            tpp = ctx.enter_context(tc.tile_pool(name="tpp", bufs=6, space="PSUM"))
            finp = ctx.enter_context(tc.tile_pool(name="finp", bufs=2, space="PSUM"))

            # constants / marshaled inputs (DMA'd once; reused every rep)
            ident = const.tile([P, 64], F32)
            nc.sync.dma_start(ident[:], ident_d[:])
            ones = const.tile([P, 1], F32)
            nc.sync.dma_start(ones[:], ones_d[:])
            g3 = const.tile([P, NT, NJ], F32)
            nc.sync.dma_start(g3[:], g_d[:])
            gm3 = const.tile([P, NT, NJ], F32)
            nc.sync.dma_start(gm3[:], gm_d[:])
            hostm = const.tile([P, NT], F32)
            nc.sync.dma_start(hostm[:], hostm_d[:])
            l0 = const.tile([P, NT], F32)
            nc.sync.dma_start(l0[:], l0_d[:])
            mval = const.tile([P, NT], F32)
            nc.sync.dma_start(mval[:], mval_d[:])
            epw = const.tile([P, NT], F32)
            nc.sync.dma_start(epw[:], epw_d[:])
            ehw = const.tile([P, NT], F32)
            nc.sync.dma_start(ehw[:], ehw_d[:])
            mvt = const.tile([64, BPC], F32)
            nc.sync.dma_start(mvt[:], mvt_d[:])
            wcat = const.tile([P, 2 * NT], F32)
            nc.sync.dma_start(wcat[:], wcat_d[:])

            def trace_stream():
                """Issue one rep's Z-pass; return tiles the epilogue needs."""
                # gmax over valid targets: no dependency on Z -> runs early
                gmx = small.tile([P, NT], F32, tag="gmx")
                nc.vector.tensor_reduce(gmx[:], gm3[:],
                                        axis=mybir.AxisListType.X, op=OP.max)
                xt4 = xp.tile([P, NT, V], F8, tag="xt")
                nc.sync.dma_start(xt4[:], x_d[:])
                zA = small.tile([P, NT], F32, tag="zA")
                zB = small.tile([P, NT], F32, tag="zB")
                for i in range(NT):
                    eta = epool.tile([P, VA], F16, tag="eta")
                    nc.scalar.activation(eta[:], xt4[:, i, 0:VA], AF.Exp,
                                         accum_out=zA[:, i:i + 1])
                    it32 = ipool.tile([P, VB], I32, tag="it")
                    nc.vector.tensor_scalar(
                        out=it32[:], in0=xt4[:, i, VA:V], scalar1=SCH_S,
                        scalar2=SCH_B, op0=OP.mult, op1=OP.add)
                    st = spool.tile([P, VB], F32, tag="st")
                    nc.vector.tensor_scalar(
                        out=st[:], in0=it32[:].bitcast(F32), scalar1=0.0,
                        scalar2=None, op0=OP.add, op1=OP.add,
                        accum_out=zB[:, i:i + 1])
                return gmx, zA, zB

            def trace_epilogue(sv):
                gmx, zA, zB = sv
                zcol = small.tile([P, NT], F32, tag="zcol")
                nc.vector.tensor_add(zcol[:], zA[:], zB[:])
                lz = small.tile([P, NT], F32, tag="lz")
                nc.scalar.activation(lz[:], zcol[:], AF.Ln)
                # lzm = -logZ - BIG*invalid_row   (hostm = 0 / -BIG)
                lzm = small.tile([P, NT], F32, tag="lzm")
                nc.vector.scalar_tensor_tensor(
                    out=lzm[:], in0=lz[:], scalar=-1.0, in1=hostm[:],
                    op0=OP.mult, op1=OP.add)
                ecat = small.tile([P, 2 * NT], F32, tag="ecat")
                nc.vector.tensor_add(ecat[:, 0:NT], gmx[:], lzm[:])
                # logp0 = l0 - logZ (unmasked)
                nc.vector.scalar_tensor_tensor(
                    out=ecat[:, NT:2 * NT], in0=lz[:], scalar=-1.0, in1=l0[:],
                    op0=OP.mult, op1=OP.add)
                pcat = small.tile([P, 2 * NT], F32, tag="pcat")
                nc.scalar.activation(pcat[:], ecat[:], AF.Exp)  # [m2 | p0]
                onem = small.tile([P, NT], F32, tag="onem")
                nc.vector.tensor_scalar(
                    out=onem[:], in0=pcat[:, NT:2 * NT], scalar1=-1.0,
                    scalar2=1.0, op0=OP.mult, op1=OP.add)       # 1 - p0
                # log1m overwrites ecat[:, 0:NT] (e1 already consumed by Exp)
                # -> ecat becomes [log1m | logp0], multiplied by wcat=[ehw|epw]
                nc.scalar.activation(ecat[:, 0:NT], onem[:], AF.Ln)
                c0t = small.tile([P, NT], F32, tag="c0t")
                nc.vector.tensor_scalar_mul(c0t[:], onem[:], C0)  # C0*(1-p0)
                tmp2 = small.tile([P, NT], F32, tag="tmp2")
                # tmp2 = C0*(1-p0) - (C0-C1)*m2
                nc.vector.scalar_tensor_tensor(
                    out=tmp2[:], in0=pcat[:, 0:NT], scalar=-(C0 - C1),
                    in1=c0t[:], op0=OP.mult, op1=OP.add)
                rcat = small.tile([P, 3 * NT], F32, tag="rcat")
                nc.vector.tensor_mul(rcat[:, 0:NT], tmp2[:], mval[:])
                nc.vector.tensor_mul(rcat[:, NT:3 * NT], ecat[:], wcat[:])

                # ---- term1: cross-row max of a = g - logZ_row - BIG*invalid
                m1all = small.tile([64, BPC], F32, tag="m1all")
                for i in range(NT):
                    at = apool.tile([P, NJ], F32, tag="at")
                    nc.vector.tensor_scalar_add(at[:], g3[:, i, :],
                                                scalar1=lzm[:, i:i + 1])
                    for h in range(2):
                        b = 2 * i + h
                        tp = tpp.tile([64, 64], F32, tag="tp")
                        nc.tensor.transpose(tp[:],
                                            at[64 * h:64 * h + 64, :],
                                            ident[64 * h:64 * h + 64, :])
                        nc.vector.tensor_reduce(m1all[:, b:b + 1], tp[:],
                                                axis=mybir.AxisListType.X,
                                                op=OP.max)
                # t1 = clamp(-amax1, C1, C0) * valid_n
                t1a = small.tile([64, BPC], F32, tag="t1a")
                nc.vector.tensor_scalar(
                    out=t1a[:], in0=m1all[:], scalar1=-1.0, scalar2=C1,
                    op0=OP.mult, op1=OP.max)
                t1b = small.tile([64, BPC], F32, tag="t1b")
                nc.vector.tensor_scalar_min(t1b[:], t1a[:], C0)
                t1col = small.tile([64, BPC], F32, tag="t1col")
                nc.vector.tensor_mul(t1col[:], t1b[:], mvt[:])
                # pair-fold 8 batch cols to NT so the sum can ride psA's cols
                t1p = small.tile([64, NT], F32, tag="t1p")
                nc.vector.tensor_add(t1p[:], t1col[:, 0:NT], t1col[:, NT:BPC])

                # ---- final partition-dim sums via matmul with ones ----
                psA = finp.tile([1, 3 * NT], F32, tag="psA")
                nc.tensor.matmul(out=psA[:], lhsT=ones[:], rhs=rcat[:],
                                 start=True, stop=False)
                nc.tensor.matmul(out=psA[0:1, 0:NT], lhsT=ones[0:64, :],
                                 rhs=t1p[:], start=False, stop=True)
                out_t = small.tile([1, 2], F32, tag="out_t")
                nc.vector.tensor_reduce(out_t[:, 0:1], psA[0:1, 0:NT],
                                        axis=mybir.AxisListType.X, op=OP.add)
                nc.vector.tensor_reduce(out_t[:, 1:2], psA[0:1, NT:3 * NT],
                                        axis=mybir.AxisListType.X, op=OP.add)
                nc.sync.dma_start(out_d[:], out_t[:])

            prev = None
            for rep in range(reps):
                cur = trace_stream()
                if prev is not None:
                    trace_epilogue(prev)
                prev = cur
            trace_epilogue(prev)

    nc.compile()
    return nc


def _prep_core_inputs(logits, targets, core):
    """Host-side marshaling for one core (batches core*BPC .. core*BPC+BPC-1)."""
    import ml_dtypes
    b0 = core * BPC
    lg = np.asarray(logits[b0:b0 + BPC], dtype=np.float32)  # [BPC, T, V]
    # [P, NT*V] fp8: x[p, i*V + v] = logit of row i*128+p, col v
    x = np.ascontiguousarray(
        lg.reshape(NT, P, V).transpose(1, 0, 2).reshape(P, NT * V)
    ).astype(ml_dtypes.float8_e4m3)
    tg = np.asarray(targets[b0:b0 + BPC])
    valid = (tg != 0) & (tg != PAD)                         # [BPC, T]
    tgc = np.where(valid, tg, 0).astype(np.int64)
    validf = valid.astype(np.float32)
    ep = (tg == 0).astype(np.float32)
    ep_w = -0.5 / (B * (ep.sum(axis=1) + EPS))              # [BPC]
    eh_w = -0.5 / (B * (validf.sum(axis=1) + EPS))

    g = np.zeros((P, NT, NJ), dtype=np.float32)
    gm = np.zeros((P, NT, NJ), dtype=np.float32)
    hostm = np.zeros((P, NT), dtype=np.float32)
    l0 = np.zeros((P, NT), dtype=np.float32)
    mval = np.zeros((P, NT), dtype=np.float32)
    epw = np.zeros((P, NT), dtype=np.float32)
    ehw = np.zeros((P, NT), dtype=np.float32)
    p = np.arange(P)
    r = p % 64
    for i in range(NT):
        bl = 2 * i + p // 64                                # [P]
        g[:, i, :] = lg[bl[:, None], r[:, None], tgc[bl, :]]
        gm[:, i, :] = g[:, i, :] + (validf[bl, :] - 1.0) * BIG
        hostm[:, i] = (validf[bl, r] - 1.0) * BIG
        l0[:, i] = lg[bl, r, 0]
        mval[:, i] = validf[bl, r]
        epw[:, i] = ep[bl, r] * ep_w[bl]
        ehw[:, i] = validf[bl, r] * eh_w[bl]
    mvt = np.ascontiguousarray(validf.T)                    # [T=64, BPC]
    wcat = np.concatenate([ehw, epw], axis=1)               # [P, 2*NT]
    ident = np.zeros((P, 64), dtype=np.float32)
    ident[np.arange(P), np.arange(P) % 64] = 1.0
    ones = np.ones((P, 1), dtype=np.float32)
    return {"x": x, "g": g, "gm": gm, "hostm": hostm, "l0": l0, "mval": mval,
            "epw": epw, "ehw": ehw, "mvt": mvt, "wcat": wcat, "ident": ident,
            "ones": ones}


_CACHE = {}


def _get_runner():
    """Build the Bass program and a cached 8-core PJRT executable."""
    if "runner" in _CACHE:
        return _CACHE["runner"]
    import jax
    from jax.sharding import Mesh, PartitionSpec
    from jax.experimental.shard_map import shard_map
    from concourse import bass2jax

    nc = _build_program()
    bass2jax.install_neuronx_cc_hook()

    part_name = nc.partition_id_tensor.name if nc.partition_id_tensor else None
    in_names, out_names, out_avals, zero_outs = [], [], [], []
    for alloc in nc.m.functions[0].allocations:
        if not isinstance(alloc, mybir.MemoryLocationSet):
            continue
        name = alloc.memorylocations[0].name
        if alloc.kind == "ExternalInput":
            if name != part_name:
                in_names.append(name)
        elif alloc.kind == "ExternalOutput":
            out_names.append(name)
            shape = tuple(alloc.tensor_shape)
            dtype = mybir.dt.np(alloc.dtype)
            out_avals.append(jax.core.ShapedArray(shape, dtype))
            zero_outs.append(np.zeros(shape, dtype))
    n_params = len(in_names)
    all_names = in_names + out_names
    if part_name is not None:
        all_names = all_names + [part_name]

    def _body(*args):
        operands = list(args)
        if part_name is not None:
            operands.append(bass2jax.partition_id_tensor())
        outs = bass2jax._bass_exec_p.bind(
            *operands,
            out_avals=tuple(out_avals),
            in_names=tuple(all_names),
            out_names=tuple(out_names),
            lowering_input_output_aliases=(),
            sim_require_finite=True,
            sim_require_nnan=True,
            nc=nc,
        )
        return tuple(outs)

    devices = jax.devices()[:N_CORES]
    mesh = Mesh(np.asarray(devices), ("core",))
    donate = tuple(range(n_params, n_params + len(out_names)))
    sharded = jax.jit(
        shard_map(_body, mesh=mesh,
                  in_specs=(PartitionSpec("core"),) * (n_params + len(out_names)),
                  out_specs=(PartitionSpec("core"),) * len(out_names),
                  check_rep=False),
        donate_argnums=donate, keep_unused=True)

    runner = (sharded, in_names, out_names, zero_outs)
    _CACHE["runner"] = runner
    return runner


def run_device(in_maps):
    """Run the SPMD program; in_maps is a list of N_CORES dicts."""
    sharded, in_names, out_names, zero_outs = _get_runner()
    concat_in = [
        np.concatenate([in_maps[c][n] for c in range(N_CORES)], axis=0)
        for n in in_names
    ]
    concat_zero = [
        np.zeros((N_CORES * z.shape[0], *z.shape[1:]), z.dtype) for z in zero_outs
    ]
    out_arrs = sharded(*concat_in, *concat_zero)
    out0 = np.asarray(out_arrs[0]).reshape(N_CORES, 1, 2)
    return out0


def kernel(logits, targets):
    logits = np.asarray(logits)
    targets = np.asarray(targets)
    in_maps = [_prep_core_inputs(logits, targets, c) for c in range(N_CORES)]
    outs = run_device(in_maps)                             # [N_CORES, 1, 2]
    label = outs[:, 0, 0].sum(dtype=np.float64)
    eos = outs[:, 0, 1].sum(dtype=np.float64)
    return (np.float32(label), np.float32(eos))


# revision 21
# speedup vs baseline: 3.5318x; 1.1313x over previous
"""Chamfer-distance criterion kernel for Trainium2 (8 NeuronCores, data-parallel over batch).

Math: the reference's two [B,T,T] pairwise cross-entropy GEMMs collapse exactly
because one side of each GEMM is a (masked) one-hot:

  probs = softmax(logits); p0 = probs[:,:,0]; valid = (t!=0)&(t!=PAD)
  knn_ce(one_hot, xs) = sum_{valid n} clamp(-amax1_n, C1, C0)
     with amax1_n = max_{valid m} (l[m, t_n] - logZ_m)
  knn_ce(xs, one_hot) = sum_n valid_n*(C0*(1-p0) - (C0-C1)*exp(gmax_n - logZ_n))
     with gmax_n = max_{valid j} l[n, t_j], C0 = -log(eps), C1 = -log1p(-(D-1)eps)

So each core needs, per row, only Z = sum(exp(l)) over the full vocab, plus
tiny reductions over host-gathered f32 logit columns at the <=64 target ids
per batch (max commutes with exp; -log(clip(exp(a))) == clamp(-a, C1, C0)).

The Z pass streams the logits as fp8_e4m3 (host cast; ~0.2% rms Z error, far
inside the 2e-2 gate) and splits the vocab between two engines per row-tile:
  cols [0, VA):   ACT exp (double-rate at fp8) + hardware row-accumulate
  cols [VA, V):   DVE Schraudolph exp -- i32(x*2^23/ln2 + B) bitcast to f32
                  IS ~exp(x); one tensor_scalar convert + one accum-sum
"""

import math
import os
import numpy as np
from contextlib import ExitStack

import concourse.bass as bass
import concourse.tile as tile
from concourse import bacc, mybir

# ---- problem constants (hardcoded per contract) ----
B, T, V = 64, 64, 8192
PAD = 8192
EPS = 1e-8
D = V - 1
C0 = float(-math.log(EPS))
C1 = float(-math.log1p(-(D - 1) * EPS))
BIG = 50.0                 # additive log-domain mask; BIG > C0 + max|l| + max logZ

N_CORES = 8
BPC = B // N_CORES          # batches per core = 8
ROWS = BPC * T              # rows per core = 512
P = 128                     # partitions per tile
NT = ROWS // P              # row tiles per core = 4
NJ = 64                     # gather slots: one per target position
VA = int(os.environ.get("KVA", "6016"))   # ACT's column share (mult of 64)
VB = V - VA                               # DVE's column share
SCH_S = float(2.0 ** 23 / math.log(2.0))  # Schraudolph scale
SCH_B = float(127.0 * 2 ** 23 - 486411.0)  # Schraudolph bias (mean-centered)
F32 = mybir.dt.float32
F16 = mybir.dt.float16
F8 = mybir.dt.float8e4
I32 = mybir.dt.int32


def _build_program(reps=1):
    nc = bacc.Bacc("TRN2", target_bir_lowering=False, debug=False)
    x_d = nc.dram_tensor("x", [P, NT * V], F8, kind="ExternalInput").ap()
    g_d = nc.dram_tensor("g", [P, NT, NJ], F32, kind="ExternalInput").ap()
    gm_d = nc.dram_tensor("gm", [P, NT, NJ], F32, kind="ExternalInput").ap()
    hostm_d = nc.dram_tensor("hostm", [P, NT], F32, kind="ExternalInput").ap()
    l0_d = nc.dram_tensor("l0", [P, NT], F32, kind="ExternalInput").ap()
    mval_d = nc.dram_tensor("mval", [P, NT], F32, kind="ExternalInput").ap()
    epw_d = nc.dram_tensor("epw", [P, NT], F32, kind="ExternalInput").ap()
    ehw_d = nc.dram_tensor("ehw", [P, NT], F32, kind="ExternalInput").ap()
    mvt_d = nc.dram_tensor("mvt", [64, BPC], F32, kind="ExternalInput").ap()
    wcat_d = nc.dram_tensor("wcat", [P, 2 * NT], F32, kind="ExternalInput").ap()
    ident_d = nc.dram_tensor("ident", [P, 128], F32, kind="ExternalInput").ap()
    ones_d = nc.dram_tensor("ones", [P, 1], F32, kind="ExternalInput").ap()
    out_d = nc.dram_tensor("out", [1, 2], F32, kind="ExternalOutput").ap()

    AF = mybir.ActivationFunctionType
    OP = mybir.AluOpType

    with tile.TileContext(nc) as tc:
        with ExitStack() as ctx:
            const = ctx.enter_context(tc.tile_pool(name="const", bufs=1))
            xp = ctx.enter_context(tc.tile_pool(name="xp", bufs=2))
            epool = ctx.enter_context(tc.tile_pool(name="ep", bufs=2))
            ipool = ctx.enter_context(tc.tile_pool(name="ip", bufs=2))
            spool = ctx.enter_context(tc.tile_pool(name="sp", bufs=2))
            apool = ctx.enter_context(tc.tile_pool(name="apool", bufs=2))
            small = ctx.enter_context(tc.tile_pool(name="small", bufs=2))
            tpp = ctx.enter_context(tc.tile_pool(name="tpp", bufs=6, space="PSUM"))
            finp = ctx.enter_context(tc.tile_pool(name="finp", bufs=2, space="PSUM"))

            # constants / marshaled inputs (DMA'd once; reused every rep)
            ident = const.tile([P, 128], F32)
            nc.sync.dma_start(ident[:], ident_d[:])
            ones = const.tile([P, 1], F32)
            nc.sync.dma_start(ones[:], ones_d[:])
            g3 = const.tile([P, NT, NJ], F32)
            nc.sync.dma_start(g3[:], g_d[:])
            gm3 = const.tile([P, NT, NJ], F32)
            nc.sync.dma_start(gm3[:], gm_d[:])
            hostm = const.tile([P, NT], F32)
            nc.sync.dma_start(hostm[:], hostm_d[:])
            l0 = const.tile([P, NT], F32)
            nc.sync.dma_start(l0[:], l0_d[:])
            mval = const.tile([P, NT], F32)
            nc.sync.dma_start(mval[:], mval_d[:])
            epw = const.tile([P, NT], F32)
            nc.sync.dma_start(epw[:], epw_d[:])
            ehw = const.tile([P, NT], F32)
            nc.sync.dma_start(ehw[:], ehw_d[:])
            mvt = const.tile([64, BPC], F32)
            nc.sync.dma_start(mvt[:], mvt_d[:])
            wcat = const.tile([P, 2 * NT], F32)
            nc.sync.dma_start(wcat[:], wcat_d[:])

            def trace_stream():
                """Issue one rep's Z-pass; return tiles the epilogue needs."""
                # gmax over valid targets: no dependency on Z -> runs early
                gmx = small.tile([P, NT], F32, tag="gmx")
                nc.vector.tensor_reduce(gmx[:], gm3[:],
                                        axis=mybir.AxisListType.X, op=OP.max)
                xt4 = xp.tile([P, NT, V], F8, tag="xt")
                nc.sync.dma_start(xt4[:], x_d[:])
                zA = small.tile([P, NT], F32, tag="zA")
                zB = small.tile([P, NT], F32, tag="zB")
                for i in range(NT):
                    eta = epool.tile([P, VA], F16, tag="eta")
                    nc.scalar.activation(eta[:], xt4[:, i, 0:VA], AF.Exp,
                                         accum_out=zA[:, i:i + 1])
                    it32 = ipool.tile([P, VB], I32, tag="it")
                    nc.vector.tensor_scalar(
                        out=it32[:], in0=xt4[:, i, VA:V], scalar1=SCH_S,
                        scalar2=SCH_B, op0=OP.mult, op1=OP.add)
                    st = spool.tile([P, VB], F32, tag="st")
                    nc.vector.tensor_scalar(
                        out=st[:], in0=it32[:].bitcast(F32), scalar1=0.0,
                        scalar2=None, op0=OP.add, op1=OP.add,
                        accum_out=zB[:, i:i + 1])
                return gmx, zA, zB

            def trace_epilogue(sv):
                gmx, zA, zB = sv
                zcol = small.tile([P, NT], F32, tag="zcol")
                nc.vector.tensor_add(zcol[:], zA[:], zB[:])
                lz = small.tile([P, NT], F32, tag="lz")
                nc.scalar.activation(lz[:], zcol[:], AF.Ln)
                # lzm = -logZ - BIG*invalid_row   (hostm = 0 / -BIG)
                lzm = small.tile([P, NT], F32, tag="lzm")
                nc.vector.scalar_tensor_tensor(
                    out=lzm[:], in0=lz[:], scalar=-1.0, in1=hostm[:],
                    op0=OP.mult, op1=OP.add)
                ecat = small.tile([P, 2 * NT], F32, tag="ecat")
                nc.vector.tensor_add(ecat[:, 0:NT], gmx[:], lzm[:])
                # logp0 = l0 - logZ (unmasked)
                nc.vector.scalar_tensor_tensor(
                    out=ecat[:, NT:2 * NT], in0=lz[:], scalar=-1.0, in1=l0[:],
                    op0=OP.mult, op1=OP.add)
                pcat = small.tile([P, 2 * NT], F32, tag="pcat")
                nc.scalar.activation(pcat[:], ecat[:], AF.Exp)  # [m2 | p0]
                onem = small.tile([P, NT], F32, tag="onem")
                nc.vector.tensor_scalar(
                    out=onem[:], in0=pcat[:, NT:2 * NT], scalar1=-1.0,
                    scalar2=1.0, op0=OP.mult, op1=OP.add)       # 1 - p0
                # log1m overwrites ecat[:, 0:NT] (e1 already consumed by Exp)
                # -> ecat becomes [log1m | logp0], multiplied by wcat=[ehw|epw]
                nc.scalar.activation(ecat[:, 0:NT], onem[:], AF.Ln)
                tmp2 = small.tile([P, NT], F32, tag="tmp2")
                # tmp2 = (1-p0) - ((C0-C1)/C0)*m2  (C0 folded into mvalC0)
                nc.vector.scalar_tensor_tensor(
                    out=tmp2[:], in0=pcat[:, 0:NT], scalar=-(C0 - C1) / C0,
                    in1=onem[:], op0=OP.mult, op1=OP.add)
                rcat = small.tile([P, 3 * NT], F32, tag="rcat")
                nc.vector.tensor_mul(rcat[:, 0:NT], tmp2[:], mval[:])
                nc.vector.tensor_mul(rcat[:, NT:3 * NT], ecat[:], wcat[:])

                # ---- term1: cross-row max of a = g - logZ_row - BIG*invalid
                m1all = small.tile([64, BPC], F32, tag="m1all")
                for i in range(NT):
                    at = apool.tile([P, NJ], F32, tag="at")
                    nc.vector.tensor_scalar_add(at[:], g3[:, i, :],
                                                scalar1=lzm[:, i:i + 1])
                    tp = tpp.tile([64, 2, 64], F32, tag="tp")
                    nc.tensor.transpose(tp[:], at[:], ident[:])
                    nc.vector.tensor_reduce(m1all[:, 2 * i:2 * i + 2], tp[:],
                                            axis=mybir.AxisListType.X,
                                            op=OP.max)
                # t1 = clamp(-amax1, C1, C0); then row-sum of t1*valid
                t1a = small.tile([64, BPC], F32, tag="t1a")
                nc.vector.tensor_scalar(
                    out=t1a[:], in0=m1all[:], scalar1=-1.0, scalar2=C1,
                    op0=OP.mult, op1=OP.max)
                t1b = small.tile([64, BPC], F32, tag="t1b")
                nc.vector.tensor_scalar_min(t1b[:], t1a[:], C0)
                t1col = small.tile([64, BPC], F32, tag="t1col")
                nc.vector.tensor_mul(t1col[:], t1b[:], mvt[:])
                t1s = small.tile([64, NT], F32, tag="t1s")
                nc.vector.tensor_add(t1s[:], t1col[:, 0:NT], t1col[:, NT:BPC])

                # ---- final partition-dim sums via matmul with ones ----
                psA = finp.tile([1, 3 * NT], F32, tag="psA")
                nc.tensor.matmul(out=psA[:], lhsT=ones[:], rhs=rcat[:],
                                 start=True, stop=False)
                nc.tensor.matmul(out=psA[0:1, 0:NT], lhsT=ones[0:64, :],
                                 rhs=t1s[:], start=False, stop=True)
                out_t = small.tile([1, 2], F32, tag="out_t")
                nc.vector.tensor_reduce(out_t[:, 0:1], psA[0:1, 0:NT],
                                        axis=mybir.AxisListType.X, op=OP.add)
                nc.vector.tensor_reduce(out_t[:, 1:2], psA[0:1, NT:3 * NT],
                                        axis=mybir.AxisListType.X, op=OP.add)
                nc.sync.dma_start(out_d[:], out_t[:])

            prev = None
            for rep in range(reps):
                cur = trace_stream()
                if prev is not None:
                    trace_epilogue(prev)
                prev = cur
            trace_epilogue(prev)

    nc.compile()
    return nc


def _prep_core_inputs(logits, targets, core):
    """Host-side marshaling for one core (batches core*BPC .. core*BPC+BPC-1)."""
    import ml_dtypes
    b0 = core * BPC
    lg = np.asarray(logits[b0:b0 + BPC], dtype=np.float32)  # [BPC, T, V]
    # [P, NT*V] fp8: x[p, i*V + v] = logit of row i*128+p, col v
    x = np.ascontiguousarray(
        lg.reshape(NT, P, V).transpose(1, 0, 2).reshape(P, NT * V)
    ).astype(ml_dtypes.float8_e4m3)
    tg = np.asarray(targets[b0:b0 + BPC])
    valid = (tg != 0) & (tg != PAD)                         # [BPC, T]
    tgc = np.where(valid, tg, 0).astype(np.int64)
    validf = valid.astype(np.float32)
    ep = (tg == 0).astype(np.float32)
    ep_w = -0.5 / (B * (ep.sum(axis=1) + EPS))              # [BPC]
    eh_w = -0.5 / (B * (validf.sum(axis=1) + EPS))

    g = np.zeros((P, NT, NJ), dtype=np.float32)
    gm = np.zeros((P, NT, NJ), dtype=np.float32)
    hostm = np.zeros((P, NT), dtype=np.float32)
    l0 = np.zeros((P, NT), dtype=np.float32)
    mval = np.zeros((P, NT), dtype=np.float32)
    epw = np.zeros((P, NT), dtype=np.float32)
    ehw = np.zeros((P, NT), dtype=np.float32)
    p = np.arange(P)
    r = p % 64
    for i in range(NT):
        bl = 2 * i + p // 64                                # [P]
        g[:, i, :] = lg[bl[:, None], r[:, None], tgc[bl, :]]
        gm[:, i, :] = g[:, i, :] + (validf[bl, :] - 1.0) * BIG
        hostm[:, i] = (validf[bl, r] - 1.0) * BIG
        l0[:, i] = lg[bl, r, 0]
        mval[:, i] = validf[bl, r] * C0
        epw[:, i] = ep[bl, r] * ep_w[bl]
        ehw[:, i] = validf[bl, r] * eh_w[bl]
    mvt = np.ascontiguousarray(validf.T)                    # [T=64, BPC]
    wcat = np.concatenate([ehw, epw], axis=1)               # [P, 2*NT]
    ident = np.zeros((P, P), dtype=np.float32)
    ident[np.arange(P), np.arange(P)] = 1.0
    ones = np.ones((P, 1), dtype=np.float32)
    return {"x": x, "g": g, "gm": gm, "hostm": hostm, "l0": l0, "mval": mval,
            "epw": epw, "ehw": ehw, "mvt": mvt, "wcat": wcat, "ident": ident,
            "ones": ones}


_CACHE = {}


def _get_runner():
    """Build the Bass program and a cached 8-core PJRT executable."""
    if "runner" in _CACHE:
        return _CACHE["runner"]
    import jax
    from jax.sharding import Mesh, PartitionSpec
    from jax.experimental.shard_map import shard_map
    from concourse import bass2jax

    nc = _build_program()
    bass2jax.install_neuronx_cc_hook()

    part_name = nc.partition_id_tensor.name if nc.partition_id_tensor else None
    in_names, out_names, out_avals, zero_outs = [], [], [], []
    for alloc in nc.m.functions[0].allocations:
        if not isinstance(alloc, mybir.MemoryLocationSet):
            continue
        name = alloc.memorylocations[0].name
        if alloc.kind == "ExternalInput":
            if name != part_name:
                in_names.append(name)
        elif alloc.kind == "ExternalOutput":
            out_names.append(name)
            shape = tuple(alloc.tensor_shape)
            dtype = mybir.dt.np(alloc.dtype)
            out_avals.append(jax.core.ShapedArray(shape, dtype))
            zero_outs.append(np.zeros(shape, dtype))
    n_params = len(in_names)
    all_names = in_names + out_names
    if part_name is not None:
        all_names = all_names + [part_name]

    def _body(*args):
        operands = list(args)
        if part_name is not None:
            operands.append(bass2jax.partition_id_tensor())
        outs = bass2jax._bass_exec_p.bind(
            *operands,
            out_avals=tuple(out_avals),
            in_names=tuple(all_names),
            out_names=tuple(out_names),
            lowering_input_output_aliases=(),
            sim_require_finite=True,
            sim_require_nnan=True,
            nc=nc,
        )
        return tuple(outs)

    devices = jax.devices()[:N_CORES]
    mesh = Mesh(np.asarray(devices), ("core",))
    donate = tuple(range(n_params, n_params + len(out_names)))
    sharded = jax.jit(
        shard_map(_body, mesh=mesh,
                  in_specs=(PartitionSpec("core"),) * (n_params + len(out_names)),
                  out_specs=(PartitionSpec("core"),) * len(out_names),
                  check_rep=False),
        donate_argnums=donate, keep_unused=True)

    runner = (sharded, in_names, out_names, zero_outs)
    _CACHE["runner"] = runner
    return runner


def run_device(in_maps):
    """Run the SPMD program; in_maps is a list of N_CORES dicts."""
    sharded, in_names, out_names, zero_outs = _get_runner()
    concat_in = [
        np.concatenate([in_maps[c][n] for c in range(N_CORES)], axis=0)
        for n in in_names
    ]
    concat_zero = [
        np.zeros((N_CORES * z.shape[0], *z.shape[1:]), z.dtype) for z in zero_outs
    ]
    out_arrs = sharded(*concat_in, *concat_zero)
    out0 = np.asarray(out_arrs[0]).reshape(N_CORES, 1, 2)
    return out0


def kernel(logits, targets):
    logits = np.asarray(logits)
    targets = np.asarray(targets)
    in_maps = [_prep_core_inputs(logits, targets, c) for c in range(N_CORES)]
    outs = run_device(in_maps)                             # [N_CORES, 1, 2]
    label = outs[:, 0, 0].sum(dtype=np.float64)
    eos = outs[:, 0, 1].sum(dtype=np.float64)
    return (np.float32(label), np.float32(eos))
